# revision 13
# baseline (speedup 1.0000x reference)
"""Trainium2 Bass kernel for a 2-layer RGCN (mean aggregation) + sigmoid scoring head.

Math (per reference):
  h1 = relu( sum_r mean_{e:(dst,r)} x[src] @ W1[r] + x @ root1 + b1 )
  h2 = relu( sum_r mean_{e:(dst,r)} h1[src] @ W2[r] + h1 @ root2 + b2 )
  out = sigmoid(h2 @ Ws + bs)

Strategy (8 NeuronCores, dst-sharded), v3:
  - All on-device data in bf16 (PSUM accumulation stays fp32): PE matmuls run
    at 1 cycle/row instead of 4 (fp32), DVE one-hot builds at 2x, and every
    DMA byte count halves.  Host-verified end-to-end rel err ~9e-4 (<< 2e-2).
  - Aggregate-then-transform: per (dst-block-of-128, relation), gather source
    rows per edge, segmented-sum via one-hot matmuls accumulating in PSUM
    (AT[f, d] += X[e, f]^T @ S[e, d], S one-hot with 1/cnt folded in), then
    transform: h[d, :] += AT_r^T @ W_r accumulated over relations.
  - Split-table gathers: int16 gather indices can't span 50000 rows, so edges
    are split into src<32768 (table x[0:32768]) and src>=32768 (table view
    x[N-32768:], idx -= N-32768).  Single-row 256B/512B gathers -- no pair
    overfetch.
  - Merged slot packing (v3.1): ALL (dst-block, relation) groups of a stream
    share ONE global slot run at max-over-cores exclusive-cumsum offsets --
    just one ceil-to-128 per stream, no per-group padding.  84->89% slot
    occupancy.  A group spanning a block boundary gets one one-hot
    build per touched block; foreign slots in the block are masked to -1 in
    that build's de column.
  - Batched one-hot builds (v4): per-build DVE TensorScalarPtr was measured
    at ~5us/instr on HW (~16.7ms of the ~18.5ms device time).  Replaced by
    per-32-block pairs of plain tensor_tensor ops: eq(io_g, de_bcast) then
    *= nv_bcast, using stride-0 broadcast APs on the de/nv columns.  3310
    DVE instrs -> ~208.
  - PSUM->SBUF copies + ReLU/Sigmoid run on the scalar(ACT) engine, keeping
    DVE free for one-hot builds; gathers round-robin the 4 SWDGE queues.
  - h1 is AllGather'd (bf16) across the 8 cores between layers.
"""

import os

import numpy as np
import ml_dtypes

import concourse.bacc as bacc
import concourse.bass as bass
import concourse.mybir as mybir
import concourse.tile as tile
import concourse.bass_utils as bass_utils

F32 = mybir.dt.float32
BF16 = mybir.dt.bfloat16
I16 = mybir.dt.int16
NPBF = ml_dtypes.bfloat16

NC = 8       # cores
LO = 32768   # lo gather table covers rows [0, LO); hi table rows [N-LO, N)
BG = 16      # one-hot build batch: block-columns per DVE tensor_tensor pair


# ---------------------------------------------------------------------------
# Host-side scheduling
# ---------------------------------------------------------------------------

def _build_schedule(src, dst, etype, N, R, n_cores):
    """Partition edges by dst shard; per (dst-block, src-table stream) pack
    all R relations into one slot run at per-relation max-over-cores offsets.
    Emit the SPMD-common build list (one one-hot build per touched block per
    relation) and per-core slot arrays."""
    ND = N // n_cores                      # dst nodes per core
    NDB = (ND + 127) // 128                # dst blocks per core
    CH = NDB * R                           # (dst-block, relation) groups
    HOFF = N - LO

    seg = dst * R + etype
    cnt = np.bincount(seg, minlength=N * R)
    norm_all = (1.0 / np.maximum(cnt, 1)).astype(np.float32)[seg]

    core_of = dst // ND
    lens = np.zeros((2, n_cores, CH), np.int64)
    per_core = []
    for c in range(n_cores):
        m = core_of == c
        s = src[m]
        dl = dst[m] - c * ND
        t = etype[m]
        nv = norm_all[m]
        hi = s >= LO
        group = (dl >> 7) * R + t
        per_core.append((s, dl, nv, group, hi))
        lens[0, c] = np.bincount(group[~hi], minlength=CH)
        lens[1, c] = np.bincount(group[hi], minlength=CH)

    ml = lens.max(axis=1)                          # [2, CH] max len per group
    # one global slot run per stream: groups at exclusive-cumsum offsets,
    # only ONE ceil-to-128 per stream (no per-dst-block padding)
    off_flat = np.cumsum(ml, axis=1) - ml          # [2, CH] in-run offsets
    total = ml.sum(axis=1)                         # [2] run slot lengths
    NBLKLO = int(-(-total[0] // 128))
    NBLK = NBLKLO + int(-(-total[1] // 128))
    NSLOT = NBLK * 128
    mlv = ml.reshape(2, NDB, R)
    off_v = off_flat.reshape(2, NDB, R)

    # build (one-hot) enumeration in device order: db -> r -> st -> kb
    nkb = np.zeros((2, NDB, R), np.int64)
    colbase = np.zeros((2, CH), np.int64)
    ncol = 0
    for db in range(NDB):
        for r in range(R):
            for st in (0, 1):
                m = int(mlv[st, db, r])
                if m == 0:
                    continue
                o = int(off_v[st, db, r])
                k = (-(-(o + m) // 128)) - o // 128
                colbase[st, db * R + r] = ncol
                nkb[st, db, r] = k
                ncol += k

    colbase_flat = colbase
    # global slot base per stream: lo run first, then hi run
    slotbase_flat = np.zeros((2, CH), np.int64)
    slotbase_flat[1] = NBLKLO * 128

    cores = []
    for c in range(n_cores):
        s, dl, nv, group, hi = per_core[c]
        gidx = np.zeros(NSLOT, np.int16)
        de = np.full((128, ncol), -1.0, np.float32)
        nvs = np.zeros((128, ncol), np.float32)
        for st in (0, 1):
            msk = hi if st else ~hi
            ss, dls, nvv, g = s[msk], dl[msk], nv[msk], group[msk]
            order = np.argsort(g, kind="stable")
            ss, dls, nvv, g = ss[order], dls[order], nvv[order], g[order]
            cl = np.bincount(g, minlength=CH)
            gstart = np.zeros(CH, np.int64)
            gstart[1:] = np.cumsum(cl)[:-1]
            q = off_flat[st][g] + (np.arange(len(ss)) - gstart[g])  # in-run pos
            slot = slotbase_flat[st][g] + q
            gidx[slot] = (ss - (HOFF if st else 0)).astype(np.int16)
            col = colbase_flat[st][g] + (q // 128 - off_flat[st][g] // 128)
            de[q % 128, col] = (dls & 127).astype(np.float32)
            nvs[q % 128, col] = nvv

        idx16 = np.tile(gidx.reshape(NSLOT // 16, 16).T, (8, 1))  # [128, NSLOT/16]
        cores.append(dict(idx16=idx16, de=de, nv=nvs))

    return dict(ND=ND, NDB=NDB, CH=CH, ml=mlv, off=off_v, nkb=nkb,
                NBLKLO=NBLKLO, NBLK=NBLK, NSLOT=NSLOT, NCOL=ncol, cores=cores)


# ---------------------------------------------------------------------------
# Device program
# ---------------------------------------------------------------------------

_STAGE = int(os.environ.get("K_STAGE", "3"))  # 1=L1 only, 2=+allgather, 3=full
_NOGATHER = bool(int(os.environ.get("K_NOGATHER", "0")))
_NOONEHOT = bool(int(os.environ.get("K_NOONEHOT", "0")))


def _builds_for(sched, db, r):
    """SPMD-common list of (global block col) for (db, r), device order."""
    out = []
    for st in (0, 1):
        m = int(sched["ml"][st, db, r])
        if m == 0:
            continue
        o = int(sched["off"][st, db, r])
        kb0 = o // 128
        base = 0 if st == 0 else sched["NBLKLO"]
        for kb in range(kb0, kb0 + int(sched["nkb"][st, db, r])):
            out.append(base + kb)
    return out


def _build_program(N, F, H, O, R, n_cores, sched, G1=64, G2=64):
    ND, NDB = sched["ND"], sched["NDB"]
    NBLKLO, NBLK, NSLOT = sched["NBLKLO"], sched["NBLK"], sched["NSLOT"]
    NCOL = sched["NCOL"]
    NBLKHI = NBLK - NBLKLO
    NDP = NDB * 128  # padded dst count per core
    HOFF = N - LO

    nc = bacc.Bacc("TRN2", target_bir_lowering=False, debug=False,
                   num_devices=n_cores, num_swdge_queues=4)

    # ---- I/O ----
    x_d = nc.dram_tensor("x", [N, F], BF16, kind="ExternalInput")
    xt_d = nc.dram_tensor("xt", [128, NDP], BF16, kind="ExternalInput")
    idx_d = nc.dram_tensor("idx16", [128, NSLOT // 16], I16, kind="ExternalInput")
    de_d = nc.dram_tensor("de", [128, NCOL], BF16, kind="ExternalInput")
    nv_d = nc.dram_tensor("nv", [128, NCOL], BF16, kind="ExternalInput")
    iog_d = nc.dram_tensor("iog", [128, BG * 128], BF16, kind="ExternalInput")
    w1_d = nc.dram_tensor("w1", [128, R * H], BF16, kind="ExternalInput")
    w2_d = nc.dram_tensor("w2", [128, R * 2 * O], BF16, kind="ExternalInput")
    r1_d = nc.dram_tensor("r1", [128, H], BF16, kind="ExternalInput")
    r2_d = nc.dram_tensor("r2", [128, 2 * O], BF16, kind="ExternalInput")
    b1_d = nc.dram_tensor("b1", [1, H], BF16, kind="ExternalInput")
    b2_d = nc.dram_tensor("b2", [1, O], BF16, kind="ExternalInput")
    ws_d = nc.dram_tensor("ws", [128, 1], BF16, kind="ExternalInput")
    bs_d = nc.dram_tensor("bs", [1, 1], F32, kind="ExternalInput")
    io_d = nc.dram_tensor("iota", [128, 128], BF16, kind="ExternalInput")
    id_d = nc.dram_tensor("ident", [128, 128], BF16, kind="ExternalInput")
    sc_d = nc.dram_tensor("scores", [1, NDP], F32, kind="ExternalOutput")

    eq, mul = mybir.AluOpType.is_equal, mybir.AluOpType.mult
    ACopy = mybir.ActivationFunctionType.Copy
    ARelu = mybir.ActivationFunctionType.Relu
    ASig = mybir.ActivationFunctionType.Sigmoid

    with tile.TileContext(nc) as tc:
        with (
            tc.tile_pool(name="const", bufs=1) as cp,
            tc.tile_pool(name="dram", bufs=1, space="DRAM") as dramp,
        ):
            def load_const(d, shape, dtype=BF16):
                t = cp.tile(shape, dtype, tag=d.name)
                nc.sync.dma_start(t[:], d[:])
                return t

            idx_s = load_const(idx_d, [128, NSLOT // 16], I16)
            de_s = load_const(de_d, [128, NCOL])
            nv_s = load_const(nv_d, [128, NCOL])
            iog_s = load_const(iog_d, [128, BG * 128])
            w1_s = load_const(w1_d, [128, R * H])
            w2_s = load_const(w2_d, [128, R * 2 * O])
            r1_s = load_const(r1_d, [128, H])
            r2_s = load_const(r2_d, [128, 2 * O])
            b1_s = load_const(b1_d, [1, H])
            b2_s = load_const(b2_d, [1, O])
            ws_s = load_const(ws_d, [128, 1])
            bs_s = load_const(bs_d, [1, 1], F32)
            io_s = load_const(io_d, [128, 128])
            id_s = load_const(id_d, [128, 128])
            xt_s = load_const(xt_d, [128, NDP])
            ones1 = cp.tile([1, 128], BF16, tag="ones1")
            nc.vector.memset(ones1[:], 1.0)
            dummy = cp.tile([128, 2 * H], BF16, tag="dummy")
            if _NOGATHER:
                nc.vector.memset(dummy[:], 0.0)

            h1loc = dramp.tile([NDP, H], BF16)      # this core's h1 rows (padded)
            if bool(int(os.environ.get("K_SHARED", "1"))):
                h1full = dramp.tile([N, H], BF16, addr_space="Shared")
            else:
                h1full = dramp.tile([N, H], BF16)   # allgathered h1

            qn = [0]

            def make_gather(ringp, lo_ap, hi_ap, elem, G, bufs):
                ring = {}

                def get(b):  # b = global block column
                    if _NOGATHER:
                        return dummy, 0
                    st = 0 if b < NBLKLO else 1
                    s0 = 0 if st == 0 else NBLKLO
                    NS = NBLKLO if st == 0 else NBLKHI
                    rel = b - s0
                    cb = rel // G
                    off = (rel % G) * elem
                    key = (st, cb)
                    if key in ring:
                        return ring[key], off
                    w = min(G, NS - cb * G)
                    t = ringp.tile([128, G * elem], BF16, tag=f"xr{st}",
                                   bufs=bufs)
                    col0 = (s0 + cb * G) * 8
                    nc.gpsimd.dma_gather(
                        t[:, : w * elem].rearrange("p (g f) -> p g f", f=elem),
                        lo_ap if st == 0 else hi_ap,
                        idx_s[:, col0: col0 + w * 8],
                        w * 128,
                        w * 128,
                        elem,
                        single_packet=False,
                        queue_num=qn[0] % 4,
                    )
                    qn[0] += 1
                    ring[key] = t
                    return t, off
                return get

            def make_se(segp, tag):
                """Batched one-hot builds: one eq+mult tensor_tensor pair per
                BG consecutive build columns; get(col) returns (tile, off)."""
                cur = {}

                def get(col):
                    g0 = (col // BG) * BG
                    if cur.get("g0") != g0:
                        w = min(BG, NCOL - g0)
                        t = segp.tile([128, BG * 128], BF16, tag=tag)
                        o3 = t[:, : w * 128].rearrange("p (w o) -> p w o", o=128)
                        i3 = iog_s[:, : w * 128].rearrange("p (w o) -> p w o",
                                                           o=128)
                        nc.vector.tensor_tensor(
                            o3, i3, de_s[:, g0:g0 + w].broadcast_to([128, w, 128]),
                            op=eq)
                        nc.vector.tensor_tensor(
                            o3, o3, nv_s[:, g0:g0 + w].broadcast_to([128, w, 128]),
                            op=mul)
                        cur["g0"], cur["t"] = g0, t
                    return cur["t"], (col - g0) * 128
                return get

            # AllGather plumbing (chunk hooks kept; default single collective)
            h1f3 = h1full[:].rearrange("(c n) h -> c n h", c=n_cores)
            ag_done = [0]

            def allgather_to(db_end):
                r0, r1 = ag_done[0] * 128, min(db_end * 128, ND)
                if r1 <= r0:
                    return
                out_ap = h1full[:] if (r0 == 0 and r1 == ND) else h1f3[:, r0:r1, :]
                nc.gpsimd.collective_compute(
                    "AllGather",
                    mybir.AluOpType.bypass,
                    replica_groups=[list(range(n_cores))],
                    ins=[h1loc[r0:r1, :].opt()],
                    outs=[out_ap.opt()],
                )
                ag_done[0] = db_end

            _AGC = int(os.environ.get("K_AGCHUNK", "1"))
            ag_marks = {(NDB * (i + 1)) // _AGC for i in range(_AGC)} if _AGC > 1 \
                else {NDB}

            # =============== LAYER 1 ===============
            with (
                tc.tile_pool(name="ring1", bufs=3) as ringp,
                tc.tile_pool(name="s1", bufs=2) as sp,
                tc.tile_pool(name="at1", bufs=4) as atp,
                tc.tile_pool(name="h1sb", bufs=2) as h1p,
                tc.tile_pool(name="pat1", bufs=3, space="PSUM") as patp,
                tc.tile_pool(name="ph1", bufs=2, space="PSUM") as php,
            ):
                get1 = make_gather(ringp, x_d[0:LO, :], x_d[HOFF:N, :], F, G1, 3)
                getse1 = make_se(sp, "seg1")

                col = 0
                for db in range(NDB):
                    psum_h = php.tile([128, H], F32)
                    nc.tensor.matmul(psum_h[:], xt_s[:, db * 128:(db + 1) * 128],
                                     r1_s[:], start=True, stop=False)
                    for r in range(R):
                        builds = _builds_for(sched, db, r)
                        tot = len(builds)
                        if tot == 0:
                            continue
                        psum_at = patp.tile([128, 128], F32)
                        for k, b in enumerate(builds):
                            xr, off = get1(b)
                            if _NOONEHOT:
                                se, soff = io_s, 0
                            else:
                                se, soff = getse1(col)
                            col += 1
                            nc.tensor.matmul(psum_at[:], xr[:, off:off + F],
                                             se[:, soff:soff + 128],
                                             start=(k == 0),
                                             stop=(k == tot - 1))
                        at_sb = atp.tile([128, 128], BF16)
                        nc.scalar.activation(at_sb[:], psum_at[:], ACopy)
                        nc.tensor.matmul(psum_h[:], at_sb[:],
                                         w1_s[:, r * H:(r + 1) * H],
                                         start=False, stop=False)
                    nc.tensor.matmul(psum_h[:], ones1[:], b1_s[:],
                                     start=False, stop=True)
                    h1_sb = h1p.tile([128, H], BF16)
                    nc.scalar.activation(h1_sb[:], psum_h[:], ARelu)
                    nc.sync.dma_start(h1loc[db * 128:(db + 1) * 128, :], h1_sb[:])
                    if _STAGE >= 2 and (db + 1) in ag_marks:
                        allgather_to(db + 1)
                    if _STAGE < 3:
                        sc_sb0 = h1p.tile([1, 128], F32, tag="scdbg")
                        nc.vector.tensor_copy(sc_sb0[:], h1_sb[0:1, 0:128])
                        nc.sync.dma_start(sc_d[0:1, db * 128:(db + 1) * 128],
                                          sc_sb0[:])

            # =============== ALLGATHER h1 (any remainder) ===============
            if _STAGE >= 2:
                allgather_to(NDB)

            # =============== LAYER 2 ===============
            if _STAGE >= 3:
              with (
                  tc.tile_pool(name="ring2", bufs=2) as ringp2,
                  tc.tile_pool(name="s2", bufs=2) as sp2,
                  tc.tile_pool(name="at2", bufs=4) as atp2,
                  tc.tile_pool(name="h2sb", bufs=2) as h2p,
                  tc.tile_pool(name="misc2", bufs=2) as mp2,
                  tc.tile_pool(name="pat2lo", bufs=2, space="PSUM") as patlo,
                  tc.tile_pool(name="pat2hi", bufs=2, space="PSUM") as pathi,
                  tc.tile_pool(name="ph2", bufs=2, space="PSUM") as php2,
                  tc.tile_pool(name="pmisc", bufs=1, space="PSUM") as pmp,
              ):
                  get2 = make_gather(ringp2, h1full[0:LO, :], h1full[HOFF:N, :],
                                     H, G2, 2)
                  getse2 = make_se(sp2, "seg2")

                  col = 0
                  for db in range(NDB):
                      # root2 term needs h1^T of this dst block
                      h1row = mp2.tile([128, H], BF16, tag="h1row")
                      nc.sync.dma_start(h1row[:], h1loc[db * 128:(db + 1) * 128, :])
                      psum_h2 = php2.tile([128, O], F32)
                      h1t = []
                      for h in range(2):
                          pt = pmp.tile([128, 128], BF16, tag="ptr")
                          nc.tensor.transpose(pt[:], h1row[:, h * 128:(h + 1) * 128],
                                              id_s[:])
                          ht = mp2.tile([128, 128], BF16, tag=f"h1t{h}")
                          nc.scalar.activation(ht[:], pt[:], ACopy)
                          h1t.append(ht)
                      nc.tensor.matmul(psum_h2[:], h1t[0][:], r2_s[:, 0:O],
                                       start=True, stop=False)
                      nc.tensor.matmul(psum_h2[:], h1t[1][:], r2_s[:, O:2 * O],
                                       start=False, stop=False)

                      for r in range(R):
                          builds = _builds_for(sched, db, r)
                          tot = len(builds)
                          if tot == 0:
                              continue
                          at_lo = patlo.tile([128, 128], F32)
                          at_hi = pathi.tile([128, 128], F32)
                          for k, b in enumerate(builds):
                              xr, off = get2(b)
                              if _NOONEHOT:
                                  se, soff = io_s, 0
                              else:
                                  se, soff = getse2(col)
                              col += 1
                              st_f, sp_f = (k == 0), (k == tot - 1)
                              nc.tensor.matmul(at_lo[:], xr[:, off:off + 128],
                                               se[:, soff:soff + 128],
                                               start=st_f, stop=sp_f)
                              nc.tensor.matmul(at_hi[:], xr[:, off + 128:off + 256],
                                               se[:, soff:soff + 128],
                                               start=st_f, stop=sp_f)
                          at_sb = atp2.tile([128, 2 * 128], BF16)
                          nc.scalar.activation(at_sb[:, 0:128], at_lo[:], ACopy)
                          nc.scalar.activation(at_sb[:, 128:256], at_hi[:], ACopy)
                          for h in range(2):
                              nc.tensor.matmul(
                                  psum_h2[:], at_sb[:, h * 128:(h + 1) * 128],
                                  w2_s[:, (r * 2 + h) * O:(r * 2 + h + 1) * O],
                                  start=False, stop=False)
                      nc.tensor.matmul(psum_h2[:], ones1[:], b2_s[:],
                                       start=False, stop=True)
                      h2_sb = h2p.tile([128, O], BF16)
                      nc.scalar.activation(h2_sb[:], psum_h2[:], ARelu)

                      # head: scores = sigmoid(h2 @ Ws + bs)
                      pt2 = pmp.tile([128, 128], BF16, tag="ptr")
                      nc.tensor.transpose(pt2[:], h2_sb[:], id_s[:])
                      h2t = mp2.tile([128, 128], BF16, tag="h2t")
                      nc.scalar.activation(h2t[:], pt2[:], ACopy)
                      psc = pmp.tile([1, 128], F32, tag="psc")
                      nc.tensor.matmul(psc[:], ws_s[:], h2t[:], start=True, stop=True)
                      sc_sb = mp2.tile([1, 128], F32, tag="scsb")
                      nc.scalar.activation(sc_sb[:], psc[:], ASig,
                                           bias=bs_s[0:1, 0:1])
                      nc.sync.dma_start(sc_d[0:1, db * 128:(db + 1) * 128], sc_sb[:])

    nc.compile()
    return nc


# ---------------------------------------------------------------------------
# Entry point
# ---------------------------------------------------------------------------

def kernel(x, edge_index, edge_type, W1, root1, b1, W2, root2, b2, Ws, bs):
    x = np.ascontiguousarray(np.asarray(x, np.float32))
    ei = np.asarray(edge_index)
    et = np.asarray(edge_type).astype(np.int64)
    src, dst = ei[0].astype(np.int64), ei[1].astype(np.int64)
    W1 = np.asarray(W1, np.float32)
    root1 = np.ascontiguousarray(np.asarray(root1, np.float32))
    b1 = np.asarray(b1, np.float32)
    W2 = np.asarray(W2, np.float32)
    root2 = np.asarray(root2, np.float32)
    b2 = np.asarray(b2, np.float32)
    Ws = np.ascontiguousarray(np.asarray(Ws, np.float32))
    bs = np.asarray(bs, np.float32)

    N, F = x.shape
    R, _, H = W1.shape
    O = W2.shape[2]

    sched = _build_schedule(src, dst, et, N, R, NC)
    ND, NDB = sched["ND"], sched["NDB"]
    NDP = NDB * 128

    nc = _build_program(N, F, H, O, R, NC, sched)

    # common (replicated) inputs
    xbf = x.astype(NPBF)
    w1f = np.concatenate([W1[r] for r in range(R)], axis=1)            # [F, R*H]
    w2f = np.concatenate(
        [W2[r][h * 128:(h + 1) * 128, :] for r in range(R) for h in range(2)],
        axis=1)                                                         # [128, R*2*O]
    r2f = np.concatenate([root2[0:128, :], root2[128:256, :]], axis=1)  # [128, 2*O]
    iota = np.tile(np.arange(128, dtype=np.float32), (128, 1))
    ident = np.eye(128, dtype=np.float32)

    common = dict(
        x=np.ascontiguousarray(xbf),
        w1=np.ascontiguousarray(w1f.astype(NPBF)),
        w2=np.ascontiguousarray(w2f.astype(NPBF)),
        r1=root1.astype(NPBF), r2=np.ascontiguousarray(r2f.astype(NPBF)),
        b1=np.ascontiguousarray(b1.reshape(1, H).astype(NPBF)),
        b2=np.ascontiguousarray(b2.reshape(1, O).astype(NPBF)),
        ws=Ws.astype(NPBF), bs=np.ascontiguousarray(bs.reshape(1, 1)),
        iota=np.ascontiguousarray(iota.astype(NPBF)),
        iog=np.ascontiguousarray(np.tile(iota.astype(NPBF), (1, BG))),
        ident=ident.astype(NPBF),
    )

    in_maps = []
    for c in range(NC):
        xt = np.zeros((128, NDP), NPBF)
        xt[:, :ND] = xbf[c * ND:(c + 1) * ND].T
        m = dict(common)
        m.update(
            xt=xt,
            idx16=np.ascontiguousarray(sched["cores"][c]["idx16"]),
            de=np.ascontiguousarray(sched["cores"][c]["de"].astype(NPBF)),
            nv=np.ascontiguousarray(sched["cores"][c]["nv"].astype(NPBF)),
        )
        in_maps.append(m)

    trace = bool(int(os.environ.get("K_TRACE", "0")))
    res = bass_utils.run_bass_kernel_spmd(nc, in_maps, core_ids=list(range(NC)),
                                          trace=trace)
    global last_exec_time_ns, last_results, last_nc, last_in_maps
    last_results = res
    last_exec_time_ns = res.exec_time_ns
    last_nc = nc
    last_in_maps = in_maps
    out = np.concatenate(
        [res.results[c]["scores"][0, :ND] for c in range(NC)])
    return out.astype(np.float32)


if __name__ == "__main__":
    import reference
    inputs = {k: np.asarray(v) for k, v in reference.setup_inputs().items()}
    got = kernel(**inputs)
    exp = np.asarray(reference.reference(**{k: v for k, v in reference.setup_inputs().items()}))
    err = np.abs(got - exp).max()
    rel = np.linalg.norm(got - exp) / np.linalg.norm(exp)
    print(f"max abs err {err:.3e}  rel {rel:.3e}")



# revision 27
# speedup vs baseline: 1.6221x; 1.6221x over previous
"""Trainium2 Bass kernel for a 2-layer RGCN (mean aggregation) + sigmoid scoring head.

Math (per reference):
  h1 = relu( sum_r mean_{e:(dst,r)} x[src] @ W1[r] + x @ root1 + b1 )
  h2 = relu( sum_r mean_{e:(dst,r)} h1[src] @ W2[r] + h1 @ root2 + b2 )
  out = sigmoid(h2 @ Ws + bs)

Strategy (8 NeuronCores, dst-sharded), v3:
  - All on-device data in bf16 (PSUM accumulation stays fp32): PE matmuls run
    at 1 cycle/row instead of 4 (fp32), DVE one-hot builds at 2x, and every
    DMA byte count halves.  Host-verified end-to-end rel err ~9e-4 (<< 2e-2).
  - Aggregate-then-transform: per (dst-block-of-128, relation), gather source
    rows per edge, segmented-sum via one-hot matmuls accumulating in PSUM
    (AT[f, d] += X[e, f]^T @ S[e, d], S one-hot with 1/cnt folded in), then
    transform: h[d, :] += AT_r^T @ W_r accumulated over relations.
  - Split-table gathers: int16 gather indices can't span 50000 rows, so edges
    are split into src<32768 (table x[0:32768]) and src>=32768 (table view
    x[N-32768:], idx -= N-32768).  Single-row 256B/512B gathers -- no pair
    overfetch.
  - Merged slot packing (v3.1): ALL (dst-block, relation) groups of a stream
    share ONE global slot run at max-over-cores exclusive-cumsum offsets --
    just one ceil-to-128 per stream, no per-group padding.  84->89% slot
    occupancy.  A group spanning a block boundary gets one one-hot
    build per touched block; foreign slots in the block are masked to -1 in
    that build's de column.
  - Batched one-hot builds (v4): one eq+mult tensor_tensor pair per BG=16
    block-columns with stride-0 broadcast APs on the de/nv columns (3310
    DVE instrs -> ~416).  Controlled same-session ablations show the whole
    build pipeline costs < 0.5 ms on HW; earlier multi-ms attributions were
    relay session-mode noise.
  - Single packed input (v5): the axon relay's per-call dispatch cost scales
    with the number of input buffers (17 pre-staged inputs with a TRIVIAL
    body measured +15.5 ms/call vs 1 input, ~0.9 ms/buffer).  All constants,
    tables and int16 gather indices (bitcast) now live in ONE [1, TOT] bf16
    dram tensor sliced by fixed offsets; bs is baked in via memset.  I/O =
    1 input + 1 output.
  - PSUM->SBUF copies + ReLU/Sigmoid run on the scalar(ACT) engine, keeping
    DVE free for one-hot builds; gathers round-robin the 4 SWDGE queues.
  - h1 is AllGather'd (bf16) across the 8 cores between layers.
"""

import os

import numpy as np
import ml_dtypes

import concourse.bacc as bacc
import concourse.bass as bass
import concourse.mybir as mybir
import concourse.tile as tile
import concourse.bass_utils as bass_utils

F32 = mybir.dt.float32
BF16 = mybir.dt.bfloat16
I16 = mybir.dt.int16
NPBF = ml_dtypes.bfloat16

NC = 8       # cores
LO = 32768   # lo gather table covers rows [0, LO); hi table rows [N-LO, N)
BG = 16      # one-hot build batch: block-columns per DVE tensor_tensor pair


# ---------------------------------------------------------------------------
# Host-side scheduling
# ---------------------------------------------------------------------------

def _build_schedule(src, dst, etype, N, R, n_cores):
    """Partition edges by dst shard; per (dst-block, src-table stream) pack
    all R relations into one slot run at per-relation max-over-cores offsets.
    Emit the SPMD-common build list (one one-hot build per touched block per
    relation) and per-core slot arrays."""
    ND = N // n_cores                      # dst nodes per core
    NDB = (ND + 127) // 128                # dst blocks per core
    CH = NDB * R                           # (dst-block, relation) groups
    HOFF = N - LO

    seg = dst * R + etype
    cnt = np.bincount(seg, minlength=N * R)
    norm_all = (1.0 / np.maximum(cnt, 1)).astype(np.float32)[seg]

    core_of = dst // ND
    lens = np.zeros((2, n_cores, CH), np.int64)
    per_core = []
    for c in range(n_cores):
        m = core_of == c
        s = src[m]
        dl = dst[m] - c * ND
        t = etype[m]
        nv = norm_all[m]
        hi = s >= LO
        group = (dl >> 7) * R + t
        per_core.append((s, dl, nv, group, hi))
        lens[0, c] = np.bincount(group[~hi], minlength=CH)
        lens[1, c] = np.bincount(group[hi], minlength=CH)

    ml = lens.max(axis=1)                          # [2, CH] max len per group
    # one global slot run per stream: groups at exclusive-cumsum offsets,
    # only ONE ceil-to-128 per stream (no per-dst-block padding)
    off_flat = np.cumsum(ml, axis=1) - ml          # [2, CH] in-run offsets
    total = ml.sum(axis=1)                         # [2] run slot lengths
    NBLKLO = int(-(-total[0] // 128))
    NBLK = NBLKLO + int(-(-total[1] // 128))
    NSLOT = NBLK * 128
    mlv = ml.reshape(2, NDB, R)
    off_v = off_flat.reshape(2, NDB, R)

    # build (one-hot) enumeration in device order: db -> r -> st -> kb
    nkb = np.zeros((2, NDB, R), np.int64)
    colbase = np.zeros((2, CH), np.int64)
    ncol = 0
    for db in range(NDB):
        for r in range(R):
            for st in (0, 1):
                m = int(mlv[st, db, r])
                if m == 0:
                    continue
                o = int(off_v[st, db, r])
                k = (-(-(o + m) // 128)) - o // 128
                colbase[st, db * R + r] = ncol
                nkb[st, db, r] = k
                ncol += k

    colbase_flat = colbase
    # global slot base per stream: lo run first, then hi run
    slotbase_flat = np.zeros((2, CH), np.int64)
    slotbase_flat[1] = NBLKLO * 128

    cores = []
    for c in range(n_cores):
        s, dl, nv, group, hi = per_core[c]
        gidx = np.zeros(NSLOT, np.int16)
        de = np.full((128, ncol), -1.0, np.float32)
        nvs = np.zeros((128, ncol), np.float32)
        for st in (0, 1):
            msk = hi if st else ~hi
            ss, dls, nvv, g = s[msk], dl[msk], nv[msk], group[msk]
            order = np.argsort(g, kind="stable")
            ss, dls, nvv, g = ss[order], dls[order], nvv[order], g[order]
            cl = np.bincount(g, minlength=CH)
            gstart = np.zeros(CH, np.int64)
            gstart[1:] = np.cumsum(cl)[:-1]
            q = off_flat[st][g] + (np.arange(len(ss)) - gstart[g])  # in-run pos
            slot = slotbase_flat[st][g] + q
            gidx[slot] = (ss - (HOFF if st else 0)).astype(np.int16)
            col = colbase_flat[st][g] + (q // 128 - off_flat[st][g] // 128)
            de[q % 128, col] = (dls & 127).astype(np.float32)
            nvs[q % 128, col] = nvv

        idx16 = np.tile(gidx.reshape(NSLOT // 16, 16).T, (8, 1))  # [128, NSLOT/16]
        cores.append(dict(idx16=idx16, de=de, nv=nvs))

    return dict(ND=ND, NDB=NDB, CH=CH, ml=mlv, off=off_v, nkb=nkb,
                NBLKLO=NBLKLO, NBLK=NBLK, NSLOT=NSLOT, NCOL=ncol, cores=cores)


# ---------------------------------------------------------------------------
# Device program
# ---------------------------------------------------------------------------

_STAGE = int(os.environ.get("K_STAGE", "3"))  # 1=L1 only, 2=+allgather, 3=full
_NOGATHER = bool(int(os.environ.get("K_NOGATHER", "0")))
_NOONEHOT = bool(int(os.environ.get("K_NOONEHOT", "0")))
# se experiment: 0=real builds consumed by PE; 1=dummy builds (broadcast AP,
# matmuls read io_s); 2=dummy builds (no broadcast, plain TT)
_SEMODE = int(os.environ.get("K_SEMODE", "0"))
# 1 = host-precomputed one-hot streamed from HBM (no DVE builds at all)
_HOSTSE = bool(int(os.environ.get("K_HOSTSE", "0")))


def _builds_for(sched, db, r):
    """SPMD-common list of (global block col) for (db, r), device order."""
    out = []
    for st in (0, 1):
        m = int(sched["ml"][st, db, r])
        if m == 0:
            continue
        o = int(sched["off"][st, db, r])
        kb0 = o // 128
        base = 0 if st == 0 else sched["NBLKLO"]
        for kb in range(kb0, kb0 + int(sched["nkb"][st, db, r])):
            out.append(base + kb)
    return out


def _pack_layout(N, F, H, O, R, NDP, NSLOT, NCOL):
    """Single-input packing: every constant lives in one [1, TOT] bf16 dram
    tensor (64-elem-aligned sections).  The axon relay's per-call cost scales
    with the number of input buffers (~0.9 ms each measured), so 16 inputs
    -> 1 is a direct metric win."""
    order = [
        ("x", N * F), ("xt", 128 * NDP), ("idx16", 128 * (NSLOT // 16)),
        ("de", 128 * NCOL), ("nv", 128 * NCOL), ("iog", 128 * BG * 128),
        ("w1", 128 * R * H), ("w2", 128 * R * 2 * O),
        ("r1", 128 * H), ("r2", 128 * 2 * O),
        ("b1", H), ("b2", O), ("ws", 128),
        ("iota", 128 * 128), ("ident", 128 * 128),
    ]
    offs, off = {}, 0
    for nm, n in order:
        offs[nm] = off
        off += -(-n // 64) * 64
    return offs, off


def _build_program(N, F, H, O, R, n_cores, sched, bs_val=0.0, G1=64, G2=64):
    ND, NDB = sched["ND"], sched["NDB"]
    NBLKLO, NBLK, NSLOT = sched["NBLKLO"], sched["NBLK"], sched["NSLOT"]
    NCOL = sched["NCOL"]
    NBLKHI = NBLK - NBLKLO
    NDP = NDB * 128  # padded dst count per core
    HOFF = N - LO

    nc = bacc.Bacc("TRN2", target_bir_lowering=False, debug=False,
                   num_devices=n_cores, num_swdge_queues=4)

    # ---- I/O: ONE packed input + one output ----
    offs, TOT = _pack_layout(N, F, H, O, R, NDP, NSLOT, NCOL)
    pack_d = nc.dram_tensor("pack", [1, TOT], BF16, kind="ExternalInput")
    if _HOSTSE:
        se_d = nc.dram_tensor("sehost", [128, NCOL * 128], BF16,
                              kind="ExternalInput")
    sc_d = nc.dram_tensor("scores", [1, NDP], F32, kind="ExternalOutput")

    def pview(nm, p, c, dtype=None):
        """[p, c] view of a packed section (rows contiguous)."""
        n = p * c
        ap = pack_d[0:1, offs[nm]: offs[nm] + n]
        if dtype is not None:
            ap = ap.bitcast(dtype)
        return ap.rearrange("o (p c) -> (o p) c", c=c)

    # gather-table views (row-major [rows, F]); lo/hi split for int16 idx
    def xview(nm, r0, r1_, width):
        o = offs[nm] + r0 * width
        return pack_d[0:1, o: o + (r1_ - r0) * width].rearrange(
            "o (n f) -> (o n) f", f=width)

    eq, mul = mybir.AluOpType.is_equal, mybir.AluOpType.mult
    ACopy = mybir.ActivationFunctionType.Copy
    ARelu = mybir.ActivationFunctionType.Relu
    ASig = mybir.ActivationFunctionType.Sigmoid

    with tile.TileContext(nc) as tc:
        with (
            tc.tile_pool(name="const", bufs=1) as cp,
            tc.tile_pool(name="dram", bufs=1, space="DRAM") as dramp,
        ):
            def load_const(nm, shape, dtype=BF16):
                t = cp.tile(shape, dtype, tag=nm)
                nc.sync.dma_start(
                    t[:], pview(nm, shape[0], shape[1],
                                None if dtype == BF16 else dtype))
                return t

            idx_s = load_const("idx16", [128, NSLOT // 16], I16)
            de_s = load_const("de", [128, NCOL])
            nv_s = load_const("nv", [128, NCOL])
            iog_s = load_const("iog", [128, BG * 128])
            w1_s = load_const("w1", [128, R * H])
            w2_s = load_const("w2", [128, R * 2 * O])
            r1_s = load_const("r1", [128, H])
            r2_s = load_const("r2", [128, 2 * O])
            b1_s = load_const("b1", [1, H])
            b2_s = load_const("b2", [1, O])
            ws_s = load_const("ws", [128, 1])
            io_s = load_const("iota", [128, 128])
            id_s = load_const("ident", [128, 128])
            xt_s = load_const("xt", [128, NDP])
            ones1 = cp.tile([1, 128], BF16, tag="ones1")
            nc.vector.memset(ones1[:], 1.0)
            bs_s = cp.tile([1, 1], F32, tag="bsc")
            nc.vector.memset(bs_s[:], float(bs_val))
            dummy = cp.tile([128, 2 * H], BF16, tag="dummy")
            if _NOGATHER:
                nc.vector.memset(dummy[:], 0.0)

            h1loc = dramp.tile([NDP, H], BF16)      # this core's h1 rows (padded)
            if bool(int(os.environ.get("K_SHARED", "1"))):
                h1full = dramp.tile([N, H], BF16, addr_space="Shared")
            else:
                h1full = dramp.tile([N, H], BF16)   # allgathered h1

            qn = [0]

            def make_gather(ringp, lo_ap, hi_ap, elem, G, bufs):
                ring = {}

                def get(b):  # b = global block column
                    if _NOGATHER:
                        return dummy, 0
                    st = 0 if b < NBLKLO else 1
                    s0 = 0 if st == 0 else NBLKLO
                    NS = NBLKLO if st == 0 else NBLKHI
                    rel = b - s0
                    cb = rel // G
                    off = (rel % G) * elem
                    key = (st, cb)
                    if key in ring:
                        return ring[key], off
                    w = min(G, NS - cb * G)
                    t = ringp.tile([128, G * elem], BF16, tag=f"xr{st}",
                                   bufs=bufs)
                    col0 = (s0 + cb * G) * 8
                    nc.gpsimd.dma_gather(
                        t[:, : w * elem].rearrange("p (g f) -> p g f", f=elem),
                        lo_ap if st == 0 else hi_ap,
                        idx_s[:, col0: col0 + w * 8],
                        w * 128,
                        w * 128,
                        elem,
                        single_packet=False,
                        queue_num=qn[0] % 4,
                    )
                    qn[0] += 1
                    ring[key] = t
                    return t, off
                return get

            def make_se(segp, tag):
                """Batched one-hot builds: one eq+mult tensor_tensor pair per
                BG consecutive build columns (or a plain HBM load when the
                one-hot is precomputed on host); get(col) -> (tile, off)."""
                cur = {}

                def get(col):
                    g0 = (col // BG) * BG
                    if cur.get("g0") != g0:
                        w = min(BG, NCOL - g0)
                        t = segp.tile([128, BG * 128], BF16, tag=tag)
                        if _HOSTSE:
                            nc.sync.dma_start(t[:, : w * 128],
                                              se_d[:, g0 * 128:(g0 + w) * 128])
                            cur["g0"], cur["t"] = g0, t
                            return cur["t"], (col - g0) * 128
                        o3 = t[:, : w * 128].rearrange("p (w o) -> p w o", o=128)
                        i3 = iog_s[:, : w * 128].rearrange("p (w o) -> p w o",
                                                           o=128)
                        if _SEMODE == 2:
                            nc.vector.tensor_tensor(
                                t[:, : w * 128], iog_s[:, : w * 128],
                                iog_s[:, : w * 128], op=eq)
                            nc.vector.tensor_tensor(
                                t[:, : w * 128], t[:, : w * 128],
                                iog_s[:, : w * 128], op=mul)
                        else:
                            nc.vector.tensor_tensor(
                                o3, i3,
                                de_s[:, g0:g0 + w].broadcast_to([128, w, 128]),
                                op=eq)
                            nc.vector.tensor_tensor(
                                o3, o3,
                                nv_s[:, g0:g0 + w].broadcast_to([128, w, 128]),
                                op=mul)
                        cur["g0"], cur["t"] = g0, t
                    if _SEMODE in (1, 2):
                        return io_s, 0
                    return cur["t"], (col - g0) * 128
                return get

            # AllGather plumbing (chunk hooks kept; default single collective)
            h1f3 = h1full[:].rearrange("(c n) h -> c n h", c=n_cores)
            ag_done = [0]

            def allgather_to(db_end):
                r0, r1 = ag_done[0] * 128, min(db_end * 128, ND)
                if r1 <= r0:
                    return
                out_ap = h1full[:] if (r0 == 0 and r1 == ND) else h1f3[:, r0:r1, :]
                nc.gpsimd.collective_compute(
                    "AllGather",
                    mybir.AluOpType.bypass,
                    replica_groups=[list(range(n_cores))],
                    ins=[h1loc[r0:r1, :].opt()],
                    outs=[out_ap.opt()],
                )
                ag_done[0] = db_end

            _AGC = int(os.environ.get("K_AGCHUNK", "1"))
            ag_marks = {(NDB * (i + 1)) // _AGC for i in range(_AGC)} if _AGC > 1 \
                else {NDB}

            # =============== LAYER 1 ===============
            with (
                tc.tile_pool(name="ring1", bufs=3) as ringp,
                tc.tile_pool(name="s1", bufs=2) as sp,
                tc.tile_pool(name="at1", bufs=4) as atp,
                tc.tile_pool(name="h1sb", bufs=2) as h1p,
                tc.tile_pool(name="pat1", bufs=3, space="PSUM") as patp,
                tc.tile_pool(name="ph1", bufs=2, space="PSUM") as php,
            ):
                get1 = make_gather(ringp, xview("x", 0, LO, F),
                                   xview("x", HOFF, N, F), F, G1, 3)
                getse1 = make_se(sp, "seg1")

                col = 0
                for db in range(NDB):
                    psum_h = php.tile([128, H], F32)
                    nc.tensor.matmul(psum_h[:], xt_s[:, db * 128:(db + 1) * 128],
                                     r1_s[:], start=True, stop=False)
                    for r in range(R):
                        builds = _builds_for(sched, db, r)
                        tot = len(builds)
                        if tot == 0:
                            continue
                        psum_at = patp.tile([128, 128], F32)
                        for k, b in enumerate(builds):
                            xr, off = get1(b)
                            if _NOONEHOT:
                                se, soff = io_s, 0
                            else:
                                se, soff = getse1(col)
                            col += 1
                            nc.tensor.matmul(psum_at[:], xr[:, off:off + F],
                                             se[:, soff:soff + 128],
                                             start=(k == 0),
                                             stop=(k == tot - 1))
                        at_sb = atp.tile([128, 128], BF16)
                        nc.scalar.activation(at_sb[:], psum_at[:], ACopy)
                        nc.tensor.matmul(psum_h[:], at_sb[:],
                                         w1_s[:, r * H:(r + 1) * H],
                                         start=False, stop=False)
                    nc.tensor.matmul(psum_h[:], ones1[:], b1_s[:],
                                     start=False, stop=True)
                    h1_sb = h1p.tile([128, H], BF16)
                    nc.scalar.activation(h1_sb[:], psum_h[:], ARelu)
                    nc.sync.dma_start(h1loc[db * 128:(db + 1) * 128, :], h1_sb[:])
                    if _STAGE >= 2 and (db + 1) in ag_marks:
                        allgather_to(db + 1)
                    if _STAGE < 3:
                        sc_sb0 = h1p.tile([1, 128], F32, tag="scdbg")
                        nc.vector.tensor_copy(sc_sb0[:], h1_sb[0:1, 0:128])
                        nc.sync.dma_start(sc_d[0:1, db * 128:(db + 1) * 128],
                                          sc_sb0[:])

            # =============== ALLGATHER h1 (any remainder) ===============
            if _STAGE >= 2:
                allgather_to(NDB)

            # =============== LAYER 2 ===============
            if _STAGE >= 3:
              with (
                  tc.tile_pool(name="ring2", bufs=2) as ringp2,
                  tc.tile_pool(name="s2", bufs=2) as sp2,
                  tc.tile_pool(name="at2", bufs=4) as atp2,
                  tc.tile_pool(name="h2sb", bufs=2) as h2p,
                  tc.tile_pool(name="misc2", bufs=2) as mp2,
                  tc.tile_pool(name="pat2lo", bufs=2, space="PSUM") as patlo,
                  tc.tile_pool(name="pat2hi", bufs=2, space="PSUM") as pathi,
                  tc.tile_pool(name="ph2", bufs=2, space="PSUM") as php2,
                  tc.tile_pool(name="pmisc", bufs=1, space="PSUM") as pmp,
              ):
                  get2 = make_gather(ringp2, h1full[0:LO, :], h1full[HOFF:N, :],
                                     H, G2, 2)
                  getse2 = make_se(sp2, "seg2")

                  col = 0
                  for db in range(NDB):
                      # root2 term needs h1^T of this dst block
                      h1row = mp2.tile([128, H], BF16, tag="h1row")
                      nc.sync.dma_start(h1row[:], h1loc[db * 128:(db + 1) * 128, :])
                      psum_h2 = php2.tile([128, O], F32)
                      h1t = []
                      for h in range(2):
                          pt = pmp.tile([128, 128], BF16, tag="ptr")
                          nc.tensor.transpose(pt[:], h1row[:, h * 128:(h + 1) * 128],
                                              id_s[:])
                          ht = mp2.tile([128, 128], BF16, tag=f"h1t{h}")
                          nc.scalar.activation(ht[:], pt[:], ACopy)
                          h1t.append(ht)
                      nc.tensor.matmul(psum_h2[:], h1t[0][:], r2_s[:, 0:O],
                                       start=True, stop=False)
                      nc.tensor.matmul(psum_h2[:], h1t[1][:], r2_s[:, O:2 * O],
                                       start=False, stop=False)

                      for r in range(R):
                          builds = _builds_for(sched, db, r)
                          tot = len(builds)
                          if tot == 0:
                              continue
                          at_lo = patlo.tile([128, 128], F32)
                          at_hi = pathi.tile([128, 128], F32)
                          for k, b in enumerate(builds):
                              xr, off = get2(b)
                              if _NOONEHOT:
                                  se, soff = io_s, 0
                              else:
                                  se, soff = getse2(col)
                              col += 1
                              st_f, sp_f = (k == 0), (k == tot - 1)
                              nc.tensor.matmul(at_lo[:], xr[:, off:off + 128],
                                               se[:, soff:soff + 128],
                                               start=st_f, stop=sp_f)
                              nc.tensor.matmul(at_hi[:], xr[:, off + 128:off + 256],
                                               se[:, soff:soff + 128],
                                               start=st_f, stop=sp_f)
                          at_sb = atp2.tile([128, 2 * 128], BF16)
                          nc.scalar.activation(at_sb[:, 0:128], at_lo[:], ACopy)
                          nc.scalar.activation(at_sb[:, 128:256], at_hi[:], ACopy)
                          for h in range(2):
                              nc.tensor.matmul(
                                  psum_h2[:], at_sb[:, h * 128:(h + 1) * 128],
                                  w2_s[:, (r * 2 + h) * O:(r * 2 + h + 1) * O],
                                  start=False, stop=False)
                      nc.tensor.matmul(psum_h2[:], ones1[:], b2_s[:],
                                       start=False, stop=True)
                      h2_sb = h2p.tile([128, O], BF16)
                      nc.scalar.activation(h2_sb[:], psum_h2[:], ARelu)

                      # head: scores = sigmoid(h2 @ Ws + bs)
                      pt2 = pmp.tile([128, 128], BF16, tag="ptr")
                      nc.tensor.transpose(pt2[:], h2_sb[:], id_s[:])
                      h2t = mp2.tile([128, 128], BF16, tag="h2t")
                      nc.scalar.activation(h2t[:], pt2[:], ACopy)
                      psc = pmp.tile([1, 128], F32, tag="psc")
                      nc.tensor.matmul(psc[:], ws_s[:], h2t[:], start=True, stop=True)
                      sc_sb = mp2.tile([1, 128], F32, tag="scsb")
                      nc.scalar.activation(sc_sb[:], psc[:], ASig,
                                           bias=bs_s[0:1, 0:1])
                      nc.sync.dma_start(sc_d[0:1, db * 128:(db + 1) * 128], sc_sb[:])

    nc.compile()
    return nc


# ---------------------------------------------------------------------------
# Entry point
# ---------------------------------------------------------------------------

def _expand_se(de, nv):
    """Densify the per-column one-hot (de/nv) into [128, NCOL*128] bf16,
    bit-identical to the device eq+mult build."""
    ncol = de.shape[1]
    nvb = nv.astype(NPBF)
    out = np.zeros((128, ncol * 128), NPBF)
    j = np.arange(128, dtype=np.float32)
    for c0 in range(0, ncol, 256):
        c1 = min(c0 + 256, ncol)
        blk = np.where(de[:, c0:c1, None] == j[None, None, :],
                       nvb[:, c0:c1, None], NPBF(0))
        out[:, c0 * 128:c1 * 128] = blk.reshape(128, -1)
    return out


def kernel(x, edge_index, edge_type, W1, root1, b1, W2, root2, b2, Ws, bs):
    x = np.ascontiguousarray(np.asarray(x, np.float32))
    ei = np.asarray(edge_index)
    et = np.asarray(edge_type).astype(np.int64)
    src, dst = ei[0].astype(np.int64), ei[1].astype(np.int64)
    W1 = np.asarray(W1, np.float32)
    root1 = np.ascontiguousarray(np.asarray(root1, np.float32))
    b1 = np.asarray(b1, np.float32)
    W2 = np.asarray(W2, np.float32)
    root2 = np.asarray(root2, np.float32)
    b2 = np.asarray(b2, np.float32)
    Ws = np.ascontiguousarray(np.asarray(Ws, np.float32))
    bs = np.asarray(bs, np.float32)

    N, F = x.shape
    R, _, H = W1.shape
    O = W2.shape[2]

    sched = _build_schedule(src, dst, et, N, R, NC)
    ND, NDB = sched["ND"], sched["NDB"]
    NDP = NDB * 128

    nc = _build_program(N, F, H, O, R, NC, sched, bs_val=float(bs.reshape(-1)[0]))

    # common (replicated) inputs
    xbf = x.astype(NPBF)
    w1f = np.concatenate([W1[r] for r in range(R)], axis=1)            # [F, R*H]
    w2f = np.concatenate(
        [W2[r][h * 128:(h + 1) * 128, :] for r in range(R) for h in range(2)],
        axis=1)                                                         # [128, R*2*O]
    r2f = np.concatenate([root2[0:128, :], root2[128:256, :]], axis=1)  # [128, 2*O]
    iota = np.tile(np.arange(128, dtype=np.float32), (128, 1))
    ident = np.eye(128, dtype=np.float32)

    NSLOT, NCOL = sched["NSLOT"], sched["NCOL"]
    offs, TOT = _pack_layout(N, F, H, O, R, NDP, NSLOT, NCOL)
    common = dict(
        x=xbf,
        w1=w1f.astype(NPBF),
        w2=w2f.astype(NPBF),
        r1=root1.astype(NPBF), r2=r2f.astype(NPBF),
        b1=b1.reshape(1, H).astype(NPBF),
        b2=b2.reshape(1, O).astype(NPBF),
        ws=Ws.astype(NPBF),
        iota=iota.astype(NPBF),
        iog=np.tile(iota.astype(NPBF), (1, BG)),
        ident=ident.astype(NPBF),
    )

    in_maps = []
    for c in range(NC):
        xt = np.zeros((128, NDP), NPBF)
        xt[:, :ND] = xbf[c * ND:(c + 1) * ND].T
        m = dict(common)
        m.update(
            xt=xt,
            idx16=sched["cores"][c]["idx16"].view(NPBF),
            de=sched["cores"][c]["de"].astype(NPBF),
            nv=sched["cores"][c]["nv"].astype(NPBF),
        )
        pack = np.zeros((1, TOT), NPBF)
        for nm, arr in m.items():
            flat = np.ascontiguousarray(arr).reshape(-1)
            pack[0, offs[nm]: offs[nm] + flat.size] = flat
        mm = dict(pack=pack)
        if bool(int(os.environ.get("K_HOSTSE", "0"))):
            mm["sehost"] = _expand_se(sched["cores"][c]["de"],
                                      sched["cores"][c]["nv"])
        in_maps.append(mm)

    trace = bool(int(os.environ.get("K_TRACE", "0")))
    res = bass_utils.run_bass_kernel_spmd(nc, in_maps, core_ids=list(range(NC)),
                                          trace=trace)
    global last_exec_time_ns, last_results, last_nc, last_in_maps
    last_results = res
    last_exec_time_ns = res.exec_time_ns
    last_nc = nc
    last_in_maps = in_maps
    out = np.concatenate(
        [res.results[c]["scores"][0, :ND] for c in range(NC)])
    return out.astype(np.float32)


if __name__ == "__main__":
    import reference
    inputs = {k: np.asarray(v) for k, v in reference.setup_inputs().items()}
    got = kernel(**inputs)
    exp = np.asarray(reference.reference(**{k: v for k, v in reference.setup_inputs().items()}))
    err = np.abs(got - exp).max()
    rel = np.linalg.norm(got - exp) / np.linalg.norm(exp)
    print(f"max abs err {err:.3e}  rel {rel:.3e}")

